# revision 24
# baseline (speedup 1.0000x reference)
"""BinaryLayer kernel for Trainium2 (8 NeuronCores).

Computes out = binarize(x) @ binarize(W), binarize(t) = where(t >= 0, 1, -1),
for x: [8192, 4096] f32, W: [4096, 4096] f32.

Sharding (2D, 8 cores as 4x2 grid): core c = (i, j) with i = c // 2 (4 row
groups of x) and j = c % 2 (2 column groups of W). Each core computes a
[2048, 2048] output block from x rows [2048*i : 2048*(i+1)] and W columns
[2048*j : 2048*(j+1)].

Per-core pipeline (everything on-device):
  1. binarize x tiles to +-0.5 in fp8e4 (one DVE tensor_scalar: (x>=0)-0.5)
  2. transpose x_bin via TensorE matmuls against an fp8 identity (PSUM), so
     the contraction dim lands on partitions; evacuate to an SBUF-resident
     x^T fp8 tensor in DoubleRow [128, 2, m] plane layout
  3. binarize W k-row blocks straight into the DoubleRow [128, 2, n] layout
  4. main matmuls in fp8 DoubleRow perf mode (K=256 per instruction),
     accumulating 4096-deep dot products in fp32 PSUM
  5. evacuate PSUM with ScalarE Copy(scale=4.0) - products were (+-0.5)^2,
     so x4 restores exact integer results - and DMA out.

All values (+-0.5 operands, 0.25*integer partial sums, x4 rescale) are exact
in fp8/fp32, so the result matches the f32 reference bit-for-bit.
"""

import os

import numpy as np

import concourse.bass as bass
import concourse.tile as tile
import concourse.mybir as mybir
from concourse import bacc
from concourse.bass_utils import run_bass_kernel_spmd
from concourse.masks import make_identity

# When set, the host hands each core its x shard in blocked-transposed
# layout [M_TILES, K, 128] (each 128-row block of x transposed), so every
# per-block DMA still carries the full contraction dim and the on-device
# TensorE transpose phase is skipped entirely.
HOST_TRANSPOSE = os.environ.get("HOST_TRANSPOSE", "0") == "1"

# When set, stationary operands use DoubleRowSwInterleave layout (software
# pre-interleaved, contiguous weight reads) instead of plain DoubleRow.
SWI = os.environ.get("SWI", "0") == "1"

# When set, use bf16 operands with plain matmuls (no DoubleRow perf mode);
# K=128 per matmul, relying on FWL for cheap weight loads.
BF16_MODE = os.environ.get("BF16_MODE", "0") == "1"

F32 = mybir.dt.float32
FP8 = mybir.dt.float8e4
BF16 = mybir.dt.bfloat16
DR = mybir.MatmulPerfMode.DoubleRow
ALU = mybir.AluOpType
ACTF = mybir.ActivationFunctionType

# Full problem shape (hardcoded; the harness always calls with these).
M_FULL, K_FULL = 8192, 4096
N_FULL = 4096

# Core grid: ROW_GROUPS x COL_GROUPS = 8. r8c1 replicates W on every core
# (maximum weight-load amortization in the matmul: 8 matmuls per LDWEIGHTS);
# r4c2 halves DMA traffic at the cost of reuse 4.
ROW_GROUPS = int(os.environ.get("ROWG", "4"))
COL_GROUPS = 8 // ROW_GROUPS
M_CORE = M_FULL // ROW_GROUPS  # 2048 rows of x per core
N_CORE = N_FULL // COL_GROUPS  # 2048 cols of W per core

M_TILES = M_CORE // 128        # 16
N_TILES = N_CORE // 512        # 4
KS = K_FULL // 128             # 32 k-subtiles of 128
KB = K_FULL // 256             # 16 DoubleRow super-blocks of 256


def binarize_half(nc, out_ap, in_ap):
    """out = (in >= 0) - 0.5  ->  +-0.5 exactly (one DVE instruction)."""
    nc.vector.tensor_scalar(out_ap, in_ap, 0.0, 0.5, ALU.is_ge, ALU.subtract)


def build_nc(loop_iters=1):
    nc = bacc.Bacc("TRN2", target_bir_lowering=False, debug=False)
    x_shape = ([M_TILES, K_FULL, 128] if HOST_TRANSPOSE
               else [M_CORE, K_FULL])
    x_ap = nc.dram_tensor("x", x_shape, F32, kind="ExternalInput").ap()
    w_ap = nc.dram_tensor("w", [K_FULL, N_CORE], F32, kind="ExternalInput").ap()
    out_ap = nc.dram_tensor("out", [M_CORE, N_CORE], F32, kind="ExternalOutput").ap()

    with tile.TileContext(nc) as tc:
        if loop_iters > 1:
            # benchmarking only: repeat the (idempotent) body on-device so
            # per-iteration time can be separated from dispatch overhead
            with tc.For_i(0, loop_iters, 1):
                kernel_body(tc, out_ap, x_ap, w_ap)
        else:
            kernel_body(tc, out_ap, x_ap, w_ap)
    nc.compile()
    return nc


def kernel_body(tc, out_ap, x_ap, w_ap):
    nc = tc.nc
    import contextlib

    with contextlib.ExitStack() as ctx:
        const_pool = ctx.enter_context(tc.tile_pool(name="const", bufs=1))
        xT_pool = ctx.enter_context(tc.tile_pool(name="xT", bufs=1))
        wB_pool = ctx.enter_context(tc.tile_pool(name="wB", bufs=1))
        xf_pool = ctx.enter_context(tc.tile_pool(name="xf", bufs=2))
        xb_pool = ctx.enter_context(tc.tile_pool(name="xb", bufs=2))
        wf_pool = ctx.enter_context(tc.tile_pool(name="wf", bufs=2))
        ob_pool = ctx.enter_context(tc.tile_pool(name="ob", bufs=2))
        ps_pool = ctx.enter_context(tc.tile_pool(name="ps", bufs=8, space="PSUM"))

        # Persistent binarized operands.
        # wB (moving operand) plane layout: element (p, b, i, n) holds W_bin
        # at contraction index k = b*256 + i*128 + p.
        # xT (stationary) layout depends on mode:
        #   plain DoubleRow:  [128, KB, 2, M_CORE], (p, b, i, m) at same k
        #   SwInterleave:     [128, KB, M_TILES, 256] - per (b, mt) the 256
        #     weight columns are [A127 B127 A126 B126 ... A0 B0] where
        #     A/B = plane 0/1 and the column index is the reversed local m.
        if BF16_MODE:
            # [p, kt, c]: value at contraction index k = kt*128 + p
            xT = xT_pool.tile([128, KS, M_CORE], BF16)
            wB = wB_pool.tile([128, KS, N_CORE], BF16)
        elif SWI:
            xT = xT_pool.tile([128, KB, M_TILES, 256], FP8)
            wB = wB_pool.tile([128, KB, 2, N_CORE], FP8)
        else:
            xT = xT_pool.tile([128, KB, 2, M_CORE], FP8)
            wB = wB_pool.tile([128, KB, 2, N_CORE], FP8)

        opd = BF16 if BF16_MODE else FP8

        def emit_w_slab(nt):
            # load + binarize W columns [nt*512, (nt+1)*512) for all k
            for kt in range(KS):
                b, i = kt // 2, kt % 2
                wf = wf_pool.tile([128, 512], F32, tag="wf")
                nc.sync.dma_start(wf[:], w_ap[kt * 128:(kt + 1) * 128,
                                              nt * 512:(nt + 1) * 512])
                full = wB[:, kt, :] if BF16_MODE else wB[:, b, i, :]
                binarize_half(nc, full[:, nt * 512:(nt + 1) * 512], wf[:])

        if not HOST_TRANSPOSE:
            # first W slab before x: the PE's transpose work then overlaps
            # the x loads, and slab-0 matmuls start as soon as x^T is ready
            emit_w_slab(0)

        def emit_x_block(mt):
            # blocked-transposed load: x_ap[mt] is [K, 128]; partition = k
            xf = xf_pool.tile([128, KS, 128], F32, tag="xf")
            src = x_ap[mt].rearrange("(kt p) m -> p kt m", p=128)
            nc.sync.dma_start(xf[:], src)
            # kt-order (b, i) matches the xT free layout (b, i, m)
            binarize_half(nc, xT[:, :, :, mt * 128:(mt + 1) * 128],
                          xf[:].rearrange("p kt m -> p (kt m)"))

        if HOST_TRANSPOSE:
            assert not SWI and not BF16_MODE
            # two x blocks first so early matmuls overlap the W window, W
            # next as full-width 1MB k-rows (b-progressive, best DMA size),
            # remaining x blocks stream after
            for mt in (0, 1):
                emit_x_block(mt)
            for kt in range(KS):
                b, i = kt // 2, kt % 2
                wf = wf_pool.tile([128, N_CORE], F32, tag="wfw")
                nc.sync.dma_start(wf[:], w_ap[kt * 128:(kt + 1) * 128, :])
                binarize_half(nc, wB[:, b, i, :], wf[:])
            for mt in range(2, M_TILES):
                emit_x_block(mt)
        else:
            # fp8 identity for TensorE transposes; SWI uses the anti-diagonal
            # permutation so the interleaved store needs only positive strides
            ident_bf16 = const_pool.tile([128, 128], BF16)
            if SWI:
                nc.gpsimd.memset(ident_bf16[:], 0.0)
                nc.gpsimd.affine_select(
                    out=ident_bf16[:], in_=ident_bf16[:],
                    compare_op=ALU.not_equal, fill=1.0,
                    base=-127, pattern=[[1, 128]], channel_multiplier=1)
            else:
                make_identity(nc, ident_bf16[:])
            if BF16_MODE:
                ident = ident_bf16
            else:
                ident = const_pool.tile([128, 128], FP8)
                nc.vector.tensor_copy(ident[:], ident_bf16[:])

            # ---- Phase X: load + binarize + transpose x ----
            x_chunk = 1024
            for mt in range(M_TILES):
                xb = xb_pool.tile([128, K_FULL], opd)
                for h in range(0, K_FULL, x_chunk):
                    xf = xf_pool.tile([128, x_chunk], F32)
                    nc.sync.dma_start(xf[:], x_ap[mt * 128:(mt + 1) * 128,
                                                   h:h + x_chunk])
                    binarize_half(nc, xb[:, h:h + x_chunk], xf[:])
                # transpose 32 k-subtiles in groups of 4 (one PSUM bank per group)
                for kg in range(KS // 4):
                    ps = ps_pool.tile([128, 512], F32, tag="ps")
                    for t in range(4):
                        kt = kg * 4 + t
                        nc.tensor.matmul(ps[:, t * 128:(t + 1) * 128],
                                         xb[:, kt * 128:(kt + 1) * 128],
                                         ident[:], start=True, stop=True)
                    if BF16_MODE:
                        dst = xT[:, 4 * kg:4 * kg + 4,
                                 mt * 128:(mt + 1) * 128]
                    elif SWI:
                        # psum col t*128+c holds x_bin[127-c, k=kt*128+p];
                        # scatter to interleaved j = 2c + (t%2) of block
                        # (b = 2kg + t//2, mt)
                        dst = xT[:, 2 * kg:2 * kg + 2, mt, :].rearrange(
                            "p b (m two) -> p b two m", two=2)
                    else:
                        dst = xT[:, 2 * kg:2 * kg + 2, :,
                                 mt * 128:(mt + 1) * 128]
                    if kg % 2 == 0:
                        nc.vector.tensor_copy(dst, ps[:])
                    else:
                        nc.scalar.activation(dst, ps[:], ACTF.Copy)

        # (W slab loading is emitted around phase X; see emit_w_slab below)

        if not HOST_TRANSPOSE:
            # remaining W slabs stream in while slab-0 matmuls run
            for nt in range(1, N_TILES):
                emit_w_slab(nt)

        # ---- Main matmuls: mt-outer, b-mid, nt-inner (each stationary
        # ---- operand serves 4 consecutive matmuls)
        mm_mode = (None if BF16_MODE else
                   (mybir.MatmulPerfMode.DoubleRowSwInterleave if SWI else DR))
        k_iters = KS if BF16_MODE else KB
        for mt in range(M_TILES):
            pss = [ps_pool.tile([128, 512], F32, name=f"ps_{mt}_{nt}",
                                tag="ps") for nt in range(N_TILES)]
            for b in range(k_iters):
                if BF16_MODE:
                    lhsT = xT[:, b, mt * 128:(mt + 1) * 128]
                elif SWI:
                    lhsT = xT[:, b, mt, :]
                else:
                    lhsT = xT[:, b, :, mt * 128:(mt + 1) * 128]
                for nt in range(N_TILES):
                    rhs = (wB[:, b, nt * 512:(nt + 1) * 512] if BF16_MODE
                           else wB[:, b, :, nt * 512:(nt + 1) * 512])
                    nc.tensor.matmul(pss[nt][:], lhsT, rhs,
                                     start=(b == 0), stop=(b == k_iters - 1),
                                     perf_mode=mm_mode)
            for nt in range(N_TILES):
                ob = ob_pool.tile([128, 512], F32)
                nc.scalar.activation(ob[:], pss[nt][:], ACTF.Copy, scale=4.0)
                nc.sync.dma_start(out_ap[mt * 128:(mt + 1) * 128,
                                         nt * 512:(nt + 1) * 512], ob[:])


_NC_CACHE = None


def get_nc():
    global _NC_CACHE
    if _NC_CACHE is None:
        _NC_CACHE = build_nc()
    return _NC_CACHE


def make_in_maps(x, kernel):
    in_maps = []
    for c in range(8):
        i, j = c // COL_GROUPS, c % COL_GROUPS
        x_shard = x[i * M_CORE:(i + 1) * M_CORE, :]
        if HOST_TRANSPOSE:
            # blocked transpose: [M_TILES, K, 128]
            x_shard = x_shard.reshape(M_TILES, 128, K_FULL).transpose(0, 2, 1)
        in_maps.append({
            "x": np.ascontiguousarray(x_shard),
            "w": np.ascontiguousarray(kernel[:, j * N_CORE:(j + 1) * N_CORE]),
        })
    return in_maps


def assemble(results):
    out = np.empty((M_FULL, N_FULL), dtype=np.float32)
    for c in range(8):
        i, j = c // COL_GROUPS, c % COL_GROUPS
        out[i * M_CORE:(i + 1) * M_CORE, j * N_CORE:(j + 1) * N_CORE] = \
            results[c]["out"]
    return out


def kernel(x, kernel):
    x = np.asarray(x, dtype=np.float32)
    w = np.asarray(kernel, dtype=np.float32)
    nc = get_nc()
    res = run_bass_kernel_spmd(nc, make_in_maps(x, w), list(range(8)))
    return assemble(res.results)


# revision 27
# speedup vs baseline: 1.1667x; 1.1667x over previous
"""BinaryLayer kernel for Trainium2 (8 NeuronCores).

Computes out = binarize(x) @ binarize(W), binarize(t) = where(t >= 0, 1, -1),
for x: [8192, 4096] f32, W: [4096, 4096] f32.

Sharding (2D, 8 cores as 4x2 grid): core c = (i, j) with i = c // 2 (4 row
groups of x) and j = c % 2 (2 column groups of W). Each core computes a
[2048, 2048] output block from x rows [2048*i : 2048*(i+1)] and W columns
[2048*j : 2048*(j+1)].

Per-core pipeline (everything on-device):
  1. binarize x tiles to +-0.5 in fp8e4 (one DVE tensor_scalar: (x>=0)-0.5)
  2. transpose x_bin via TensorE matmuls against an fp8 identity (PSUM), so
     the contraction dim lands on partitions; evacuate to an SBUF-resident
     x^T fp8 tensor in DoubleRow [128, 2, m] plane layout
  3. binarize W k-row blocks straight into the DoubleRow [128, 2, n] layout
  4. main matmuls in fp8 DoubleRow perf mode (K=256 per instruction),
     accumulating 4096-deep dot products in fp32 PSUM
  5. evacuate PSUM with ScalarE Copy(scale=4.0) - products were (+-0.5)^2,
     so x4 restores exact integer results - and DMA out.

All values (+-0.5 operands, 0.25*integer partial sums, x4 rescale) are exact
in fp8/fp32, so the result matches the f32 reference bit-for-bit.
"""

import os

import numpy as np

import concourse.bass as bass
import concourse.tile as tile
import concourse.mybir as mybir
from concourse import bacc
from concourse.bass_utils import run_bass_kernel_spmd
from concourse.masks import make_identity

# When set, the host hands each core its x shard in blocked-transposed
# layout [M_TILES, K, 128] (each 128-row block of x transposed), so every
# per-block DMA still carries the full contraction dim and the on-device
# TensorE transpose phase is skipped entirely.
HOST_TRANSPOSE = os.environ.get("HOST_TRANSPOSE", "0") == "1"

# When set, stationary operands use DoubleRowSwInterleave layout (software
# pre-interleaved, contiguous weight reads) instead of plain DoubleRow.
SWI = os.environ.get("SWI", "0") == "1"

# When set, use bf16 operands with plain matmuls (no DoubleRow perf mode);
# K=128 per matmul, relying on FWL for cheap weight loads.
BF16_MODE = os.environ.get("BF16_MODE", "0") == "1"

F32 = mybir.dt.float32
FP8 = mybir.dt.float8e4
BF16 = mybir.dt.bfloat16
DR = mybir.MatmulPerfMode.DoubleRow
ALU = mybir.AluOpType
ACTF = mybir.ActivationFunctionType

# Full problem shape (hardcoded; the harness always calls with these).
M_FULL, K_FULL = 8192, 4096
N_FULL = 4096

# Core grid: ROW_GROUPS x COL_GROUPS = 8. r8c1 replicates W on every core
# (maximum weight-load amortization in the matmul: 8 matmuls per LDWEIGHTS);
# r4c2 halves DMA traffic at the cost of reuse 4.
ROW_GROUPS = int(os.environ.get("ROWG", "4"))
COL_GROUPS = 8 // ROW_GROUPS
M_CORE = M_FULL // ROW_GROUPS  # 2048 rows of x per core
N_CORE = N_FULL // COL_GROUPS  # 2048 cols of W per core

M_TILES = M_CORE // 128        # 16
N_TILES = N_CORE // 512        # 4
KS = K_FULL // 128             # 32 k-subtiles of 128
KB = K_FULL // 256             # 16 DoubleRow super-blocks of 256


def binarize_half(nc, out_ap, in_ap):
    """out = (in >= 0) - 0.5  ->  +-0.5 exactly (one DVE instruction)."""
    nc.vector.tensor_scalar(out_ap, in_ap, 0.0, 0.5, ALU.is_ge, ALU.subtract)


def build_nc(loop_iters=1):
    nc = bacc.Bacc("TRN2", target_bir_lowering=False, debug=False)
    x_shape = ([M_TILES, 128, KS, 128] if HOST_TRANSPOSE
               else [M_CORE, K_FULL])
    x_ap = nc.dram_tensor("x", x_shape, F32, kind="ExternalInput").ap()
    w_ap = nc.dram_tensor("w", [K_FULL, N_CORE], F32, kind="ExternalInput").ap()
    out_ap = nc.dram_tensor("out", [M_CORE, N_CORE], F32, kind="ExternalOutput").ap()

    with tile.TileContext(nc) as tc:
        if loop_iters > 1:
            # benchmarking only: repeat the (idempotent) body on-device so
            # per-iteration time can be separated from dispatch overhead
            with tc.For_i(0, loop_iters, 1):
                kernel_body(tc, out_ap, x_ap, w_ap)
        else:
            kernel_body(tc, out_ap, x_ap, w_ap)
    nc.compile()
    return nc


def kernel_body(tc, out_ap, x_ap, w_ap):
    nc = tc.nc
    import contextlib

    with contextlib.ExitStack() as ctx:
        const_pool = ctx.enter_context(tc.tile_pool(name="const", bufs=1))
        xT_pool = ctx.enter_context(tc.tile_pool(name="xT", bufs=1))
        wB_pool = ctx.enter_context(tc.tile_pool(name="wB", bufs=1))
        xf_pool = ctx.enter_context(tc.tile_pool(name="xf", bufs=2))
        xb_pool = ctx.enter_context(tc.tile_pool(name="xb", bufs=2))
        wf_pool = ctx.enter_context(tc.tile_pool(name="wf", bufs=2))
        ob_pool = ctx.enter_context(tc.tile_pool(name="ob", bufs=2))
        ps_pool = ctx.enter_context(tc.tile_pool(name="ps", bufs=8, space="PSUM"))

        # Persistent binarized operands.
        # wB (moving operand) plane layout: element (p, b, i, n) holds W_bin
        # at contraction index k = b*256 + i*128 + p.
        # xT (stationary) layout depends on mode:
        #   plain DoubleRow:  [128, KB, 2, M_CORE], (p, b, i, m) at same k
        #   SwInterleave:     [128, KB, M_TILES, 256] - per (b, mt) the 256
        #     weight columns are [A127 B127 A126 B126 ... A0 B0] where
        #     A/B = plane 0/1 and the column index is the reversed local m.
        if BF16_MODE:
            # [p, kt, c]: value at contraction index k = kt*128 + p
            xT = xT_pool.tile([128, KS, M_CORE], BF16)
            wB = wB_pool.tile([128, KS, N_CORE], BF16)
        elif SWI:
            xT = xT_pool.tile([128, KB, M_TILES, 256], FP8)
            wB = wB_pool.tile([128, KB, 2, N_CORE], FP8)
        else:
            xT = xT_pool.tile([128, KB, 2, M_CORE], FP8)
            wB = wB_pool.tile([128, KB, 2, N_CORE], FP8)

        opd = BF16 if BF16_MODE else FP8

        def emit_w_slab(nt):
            # load + binarize W columns [nt*512, (nt+1)*512) for all k
            for kt in range(KS):
                b, i = kt // 2, kt % 2
                wf = wf_pool.tile([128, 512], F32, tag="wf")
                nc.sync.dma_start(wf[:], w_ap[kt * 128:(kt + 1) * 128,
                                              nt * 512:(nt + 1) * 512])
                full = wB[:, kt, :] if BF16_MODE else wB[:, b, i, :]
                binarize_half(nc, full[:, nt * 512:(nt + 1) * 512], wf[:])

        if not HOST_TRANSPOSE:
            # first W slab before x: the PE's transpose work then overlaps
            # the x loads, and slab-0 matmuls start as soon as x^T is ready
            emit_w_slab(0)

        def emit_x_block(mt):
            # host pre-swizzled block: x_ap[mt] is [128p, KS, 128m] - the
            # exact SBUF image, so the DMA is one contiguous 2MB read
            xf = xf_pool.tile([128, KS, 128], F32, tag="xf")
            nc.sync.dma_start(xf[:], x_ap[mt])
            # kt-order (b, i) matches the xT free layout (b, i, m)
            binarize_half(nc, xT[:, :, :, mt * 128:(mt + 1) * 128],
                          xf[:].rearrange("p kt m -> p (kt m)"))

        if HOST_TRANSPOSE:
            assert not SWI and not BF16_MODE
            # two x blocks first so early matmuls overlap the W window, W
            # next as full-width 1MB k-rows (b-progressive, best DMA size),
            # remaining x blocks stream after
            for mt in (0, 1):
                emit_x_block(mt)
            for kt in range(KS):
                b, i = kt // 2, kt % 2
                wf = wf_pool.tile([128, N_CORE], F32, tag="wfw")
                nc.sync.dma_start(wf[:], w_ap[kt * 128:(kt + 1) * 128, :])
                binarize_half(nc, wB[:, b, i, :], wf[:])
            for mt in range(2, M_TILES):
                emit_x_block(mt)
        else:
            # fp8 identity for TensorE transposes; SWI uses the anti-diagonal
            # permutation so the interleaved store needs only positive strides
            ident_bf16 = const_pool.tile([128, 128], BF16)
            if SWI:
                nc.gpsimd.memset(ident_bf16[:], 0.0)
                nc.gpsimd.affine_select(
                    out=ident_bf16[:], in_=ident_bf16[:],
                    compare_op=ALU.not_equal, fill=1.0,
                    base=-127, pattern=[[1, 128]], channel_multiplier=1)
            else:
                make_identity(nc, ident_bf16[:])
            if BF16_MODE:
                ident = ident_bf16
            else:
                ident = const_pool.tile([128, 128], FP8)
                nc.vector.tensor_copy(ident[:], ident_bf16[:])

            # ---- Phase X: load + binarize + transpose x ----
            x_chunk = 1024
            for mt in range(M_TILES):
                xb = xb_pool.tile([128, K_FULL], opd)
                for h in range(0, K_FULL, x_chunk):
                    xf = xf_pool.tile([128, x_chunk], F32)
                    nc.sync.dma_start(xf[:], x_ap[mt * 128:(mt + 1) * 128,
                                                   h:h + x_chunk])
                    binarize_half(nc, xb[:, h:h + x_chunk], xf[:])
                # transpose 32 k-subtiles in groups of 4 (one PSUM bank per group)
                for kg in range(KS // 4):
                    ps = ps_pool.tile([128, 512], F32, tag="ps")
                    for t in range(4):
                        kt = kg * 4 + t
                        nc.tensor.matmul(ps[:, t * 128:(t + 1) * 128],
                                         xb[:, kt * 128:(kt + 1) * 128],
                                         ident[:], start=True, stop=True)
                    if BF16_MODE:
                        dst = xT[:, 4 * kg:4 * kg + 4,
                                 mt * 128:(mt + 1) * 128]
                    elif SWI:
                        # psum col t*128+c holds x_bin[127-c, k=kt*128+p];
                        # scatter to interleaved j = 2c + (t%2) of block
                        # (b = 2kg + t//2, mt)
                        dst = xT[:, 2 * kg:2 * kg + 2, mt, :].rearrange(
                            "p b (m two) -> p b two m", two=2)
                    else:
                        dst = xT[:, 2 * kg:2 * kg + 2, :,
                                 mt * 128:(mt + 1) * 128]
                    if kg % 2 == 0:
                        nc.vector.tensor_copy(dst, ps[:])
                    else:
                        nc.scalar.activation(dst, ps[:], ACTF.Copy)

        # (W slab loading is emitted around phase X; see emit_w_slab below)

        if not HOST_TRANSPOSE:
            # remaining W slabs stream in while slab-0 matmuls run
            for nt in range(1, N_TILES):
                emit_w_slab(nt)

        # ---- Main matmuls: mt-outer, b-mid, nt-inner (each stationary
        # ---- operand serves 4 consecutive matmuls)
        mm_mode = (None if BF16_MODE else
                   (mybir.MatmulPerfMode.DoubleRowSwInterleave if SWI else DR))
        k_iters = KS if BF16_MODE else KB
        for mt in range(M_TILES):
            pss = [ps_pool.tile([128, 512], F32, name=f"ps_{mt}_{nt}",
                                tag="ps") for nt in range(N_TILES)]
            for b in range(k_iters):
                if BF16_MODE:
                    lhsT = xT[:, b, mt * 128:(mt + 1) * 128]
                elif SWI:
                    lhsT = xT[:, b, mt, :]
                else:
                    lhsT = xT[:, b, :, mt * 128:(mt + 1) * 128]
                for nt in range(N_TILES):
                    rhs = (wB[:, b, nt * 512:(nt + 1) * 512] if BF16_MODE
                           else wB[:, b, :, nt * 512:(nt + 1) * 512])
                    nc.tensor.matmul(pss[nt][:], lhsT, rhs,
                                     start=(b == 0), stop=(b == k_iters - 1),
                                     perf_mode=mm_mode)
            for nt in range(N_TILES):
                ob = ob_pool.tile([128, 512], F32)
                nc.scalar.activation(ob[:], pss[nt][:], ACTF.Copy, scale=4.0)
                nc.sync.dma_start(out_ap[mt * 128:(mt + 1) * 128,
                                         nt * 512:(nt + 1) * 512], ob[:])


_NC_CACHE = None


def get_nc():
    global _NC_CACHE
    if _NC_CACHE is None:
        _NC_CACHE = build_nc()
    return _NC_CACHE


def make_in_maps(x, kernel):
    in_maps = []
    for c in range(8):
        i, j = c // COL_GROUPS, c % COL_GROUPS
        x_shard = x[i * M_CORE:(i + 1) * M_CORE, :]
        if HOST_TRANSPOSE:
            # pre-swizzle to the SBUF image [M_TILES, 128p, KS, 128m]:
            # element (mt, p, kt, m) = x[mt*128 + m, kt*128 + p]
            x_shard = x_shard.reshape(M_TILES, 128, KS, 128) \
                .transpose(0, 3, 2, 1)
        in_maps.append({
            "x": np.ascontiguousarray(x_shard),
            "w": np.ascontiguousarray(kernel[:, j * N_CORE:(j + 1) * N_CORE]),
        })
    return in_maps


def assemble(results):
    out = np.empty((M_FULL, N_FULL), dtype=np.float32)
    for c in range(8):
        i, j = c // COL_GROUPS, c % COL_GROUPS
        out[i * M_CORE:(i + 1) * M_CORE, j * N_CORE:(j + 1) * N_CORE] = \
            results[c]["out"]
    return out


def kernel(x, kernel):
    x = np.asarray(x, dtype=np.float32)
    w = np.asarray(kernel, dtype=np.float32)
    nc = get_nc()
    res = run_bass_kernel_spmd(nc, make_in_maps(x, w), list(range(8)))
    return assemble(res.results)


# revision 28
# speedup vs baseline: 1.1950x; 1.0243x over previous
"""BinaryLayer kernel for Trainium2 (8 NeuronCores).

Computes out = binarize(x) @ binarize(W), binarize(t) = where(t >= 0, 1, -1),
for x: [8192, 4096] f32, W: [4096, 4096] f32.

Sharding (2D, 8 cores as 4x2 grid): core c = (i, j) with i = c // 2 (4 row
groups of x) and j = c % 2 (2 column groups of W). Each core computes a
[2048, 2048] output block from x rows [2048*i : 2048*(i+1)] and W columns
[2048*j : 2048*(j+1)].

Per-core pipeline (everything on-device):
  1. binarize x tiles to +-0.5 in fp8e4 (one DVE tensor_scalar: (x>=0)-0.5)
  2. transpose x_bin via TensorE matmuls against an fp8 identity (PSUM), so
     the contraction dim lands on partitions; evacuate to an SBUF-resident
     x^T fp8 tensor in DoubleRow [128, 2, m] plane layout
  3. binarize W k-row blocks straight into the DoubleRow [128, 2, n] layout
  4. main matmuls in fp8 DoubleRow perf mode (K=256 per instruction),
     accumulating 4096-deep dot products in fp32 PSUM
  5. evacuate PSUM with ScalarE Copy(scale=4.0) - products were (+-0.5)^2,
     so x4 restores exact integer results - and DMA out.

All values (+-0.5 operands, 0.25*integer partial sums, x4 rescale) are exact
in fp8/fp32, so the result matches the f32 reference bit-for-bit.
"""

import os

import numpy as np

import concourse.bass as bass
import concourse.tile as tile
import concourse.mybir as mybir
from concourse import bacc
from concourse.bass_utils import run_bass_kernel_spmd
from concourse.masks import make_identity

# When set, the host hands each core its x shard in blocked-transposed
# layout [M_TILES, K, 128] (each 128-row block of x transposed), so every
# per-block DMA still carries the full contraction dim and the on-device
# TensorE transpose phase is skipped entirely.
HOST_TRANSPOSE = os.environ.get("HOST_TRANSPOSE", "0") == "1"

# When set, stationary operands use DoubleRowSwInterleave layout (software
# pre-interleaved, contiguous weight reads) instead of plain DoubleRow.
SWI = os.environ.get("SWI", "0") == "1"

# When set, use bf16 operands with plain matmuls (no DoubleRow perf mode);
# K=128 per matmul, relying on FWL for cheap weight loads.
BF16_MODE = os.environ.get("BF16_MODE", "0") == "1"

F32 = mybir.dt.float32
FP8 = mybir.dt.float8e4
BF16 = mybir.dt.bfloat16
DR = mybir.MatmulPerfMode.DoubleRow
ALU = mybir.AluOpType
ACTF = mybir.ActivationFunctionType

# Full problem shape (hardcoded; the harness always calls with these).
M_FULL, K_FULL = 8192, 4096
N_FULL = 4096

# Core grid: ROW_GROUPS x COL_GROUPS = 8. r8c1 replicates W on every core
# (maximum weight-load amortization in the matmul: 8 matmuls per LDWEIGHTS);
# r4c2 halves DMA traffic at the cost of reuse 4.
ROW_GROUPS = int(os.environ.get("ROWG", "4"))
COL_GROUPS = 8 // ROW_GROUPS
M_CORE = M_FULL // ROW_GROUPS  # 2048 rows of x per core
N_CORE = N_FULL // COL_GROUPS  # 2048 cols of W per core

M_TILES = M_CORE // 128        # 16
N_TILES = N_CORE // 512        # 4
KS = K_FULL // 128             # 32 k-subtiles of 128
KB = K_FULL // 256             # 16 DoubleRow super-blocks of 256


def binarize_half(nc, out_ap, in_ap):
    """out = (in >= 0) - 0.5  ->  +-0.5 exactly (one DVE instruction)."""
    nc.vector.tensor_scalar(out_ap, in_ap, 0.0, 0.5, ALU.is_ge, ALU.subtract)


def build_nc(loop_iters=1):
    nc = bacc.Bacc("TRN2", target_bir_lowering=False, debug=False)
    x_shape = ([M_TILES, 128, KS, 128] if HOST_TRANSPOSE
               else [M_CORE, K_FULL])
    x_ap = nc.dram_tensor("x", x_shape, F32, kind="ExternalInput").ap()
    w_ap = nc.dram_tensor("w", [K_FULL, N_CORE], F32, kind="ExternalInput").ap()
    out_ap = nc.dram_tensor("out", [M_CORE, N_CORE], F32, kind="ExternalOutput").ap()

    with tile.TileContext(nc) as tc:
        if loop_iters > 1:
            # benchmarking only: repeat the (idempotent) body on-device so
            # per-iteration time can be separated from dispatch overhead
            with tc.For_i(0, loop_iters, 1):
                kernel_body(tc, out_ap, x_ap, w_ap)
        else:
            kernel_body(tc, out_ap, x_ap, w_ap)
    nc.compile()
    return nc


def kernel_body(tc, out_ap, x_ap, w_ap):
    nc = tc.nc
    import contextlib

    with contextlib.ExitStack() as ctx:
        const_pool = ctx.enter_context(tc.tile_pool(name="const", bufs=1))
        xT_pool = ctx.enter_context(tc.tile_pool(name="xT", bufs=1))
        wB_pool = ctx.enter_context(tc.tile_pool(name="wB", bufs=1))
        xf_pool = ctx.enter_context(tc.tile_pool(name="xf", bufs=2))
        xb_pool = ctx.enter_context(tc.tile_pool(name="xb", bufs=2))
        wf_pool = ctx.enter_context(tc.tile_pool(name="wf", bufs=2))
        ob_pool = ctx.enter_context(tc.tile_pool(name="ob", bufs=2))
        ps_pool = ctx.enter_context(tc.tile_pool(name="ps", bufs=8, space="PSUM"))

        # Persistent binarized operands.
        # wB (moving operand) plane layout: element (p, b, i, n) holds W_bin
        # at contraction index k = b*256 + i*128 + p.
        # xT (stationary) layout depends on mode:
        #   plain DoubleRow:  [128, KB, 2, M_CORE], (p, b, i, m) at same k
        #   SwInterleave:     [128, KB, M_TILES, 256] - per (b, mt) the 256
        #     weight columns are [A127 B127 A126 B126 ... A0 B0] where
        #     A/B = plane 0/1 and the column index is the reversed local m.
        if BF16_MODE:
            # [p, kt, c]: value at contraction index k = kt*128 + p
            xT = xT_pool.tile([128, KS, M_CORE], BF16)
            wB = wB_pool.tile([128, KS, N_CORE], BF16)
        elif SWI:
            xT = xT_pool.tile([128, KB, M_TILES, 256], FP8)
            wB = wB_pool.tile([128, KB, 2, N_CORE], FP8)
        else:
            xT = xT_pool.tile([128, KB, 2, M_CORE], FP8)
            wB = wB_pool.tile([128, KB, 2, N_CORE], FP8)

        opd = BF16 if BF16_MODE else FP8

        def emit_w_slab(nt):
            # load + binarize W columns [nt*512, (nt+1)*512) for all k
            for kt in range(KS):
                b, i = kt // 2, kt % 2
                wf = wf_pool.tile([128, 512], F32, tag="wf")
                nc.sync.dma_start(wf[:], w_ap[kt * 128:(kt + 1) * 128,
                                              nt * 512:(nt + 1) * 512])
                full = wB[:, kt, :] if BF16_MODE else wB[:, b, i, :]
                binarize_half(nc, full[:, nt * 512:(nt + 1) * 512], wf[:])

        if not HOST_TRANSPOSE:
            # first W slab before x: the PE's transpose work then overlaps
            # the x loads, and slab-0 matmuls start as soon as x^T is ready
            emit_w_slab(0)

        def emit_x_block(mt):
            # host pre-swizzled block: x_ap[mt] is [128p, KS, 128m] - the
            # exact SBUF image, so the DMA is one contiguous 2MB read
            xf = xf_pool.tile([128, KS, 128], F32, tag="xf")
            nc.sync.dma_start(xf[:], x_ap[mt])
            # kt-order (b, i) matches the xT free layout (b, i, m)
            binarize_half(nc, xT[:, :, :, mt * 128:(mt + 1) * 128],
                          xf[:].rearrange("p kt m -> p (kt m)"))

        if HOST_TRANSPOSE:
            assert not SWI and not BF16_MODE

            def emit_w_rows():
                # full-width 1MB k-rows (best DMA size)
                for kt in range(KS):
                    b, i = kt // 2, kt % 2
                    wf = wf_pool.tile([128, N_CORE], F32, tag="wfw")
                    nc.sync.dma_start(wf[:], w_ap[kt * 128:(kt + 1) * 128, :])
                    binarize_half(nc, wB[:, b, i, :], wf[:])

            order = os.environ.get("BHT_ORDER", "x01")
            if order == "wfirst":
                emit_w_rows()
                for mt in range(M_TILES):
                    emit_x_block(mt)
            else:
                # two x blocks first so early matmuls overlap the W window
                for mt in (0, 1):
                    emit_x_block(mt)
                emit_w_rows()
                for mt in range(2, M_TILES):
                    emit_x_block(mt)
        else:
            # fp8 identity for TensorE transposes; SWI uses the anti-diagonal
            # permutation so the interleaved store needs only positive strides
            ident_bf16 = const_pool.tile([128, 128], BF16)
            if SWI:
                nc.gpsimd.memset(ident_bf16[:], 0.0)
                nc.gpsimd.affine_select(
                    out=ident_bf16[:], in_=ident_bf16[:],
                    compare_op=ALU.not_equal, fill=1.0,
                    base=-127, pattern=[[1, 128]], channel_multiplier=1)
            else:
                make_identity(nc, ident_bf16[:])
            if BF16_MODE:
                ident = ident_bf16
            else:
                ident = const_pool.tile([128, 128], FP8)
                nc.vector.tensor_copy(ident[:], ident_bf16[:])

            # ---- Phase X: load + binarize + transpose x ----
            x_chunk = 1024
            for mt in range(M_TILES):
                xb = xb_pool.tile([128, K_FULL], opd)
                for h in range(0, K_FULL, x_chunk):
                    xf = xf_pool.tile([128, x_chunk], F32)
                    nc.sync.dma_start(xf[:], x_ap[mt * 128:(mt + 1) * 128,
                                                   h:h + x_chunk])
                    binarize_half(nc, xb[:, h:h + x_chunk], xf[:])
                # transpose 32 k-subtiles in groups of 4 (one PSUM bank per group)
                for kg in range(KS // 4):
                    ps = ps_pool.tile([128, 512], F32, tag="ps")
                    for t in range(4):
                        kt = kg * 4 + t
                        nc.tensor.matmul(ps[:, t * 128:(t + 1) * 128],
                                         xb[:, kt * 128:(kt + 1) * 128],
                                         ident[:], start=True, stop=True)
                    if BF16_MODE:
                        dst = xT[:, 4 * kg:4 * kg + 4,
                                 mt * 128:(mt + 1) * 128]
                    elif SWI:
                        # psum col t*128+c holds x_bin[127-c, k=kt*128+p];
                        # scatter to interleaved j = 2c + (t%2) of block
                        # (b = 2kg + t//2, mt)
                        dst = xT[:, 2 * kg:2 * kg + 2, mt, :].rearrange(
                            "p b (m two) -> p b two m", two=2)
                    else:
                        dst = xT[:, 2 * kg:2 * kg + 2, :,
                                 mt * 128:(mt + 1) * 128]
                    if kg % 2 == 0:
                        nc.vector.tensor_copy(dst, ps[:])
                    else:
                        nc.scalar.activation(dst, ps[:], ACTF.Copy)

        # (W slab loading is emitted around phase X; see emit_w_slab below)

        if not HOST_TRANSPOSE:
            # remaining W slabs stream in while slab-0 matmuls run
            for nt in range(1, N_TILES):
                emit_w_slab(nt)

        # ---- Main matmuls: mt-outer, b-mid, nt-inner (each stationary
        # ---- operand serves 4 consecutive matmuls)
        mm_mode = (None if BF16_MODE else
                   (mybir.MatmulPerfMode.DoubleRowSwInterleave if SWI else DR))
        k_iters = KS if BF16_MODE else KB
        for mt in range(M_TILES):
            pss = [ps_pool.tile([128, 512], F32, name=f"ps_{mt}_{nt}",
                                tag="ps") for nt in range(N_TILES)]
            for b in range(k_iters):
                if BF16_MODE:
                    lhsT = xT[:, b, mt * 128:(mt + 1) * 128]
                elif SWI:
                    lhsT = xT[:, b, mt, :]
                else:
                    lhsT = xT[:, b, :, mt * 128:(mt + 1) * 128]
                for nt in range(N_TILES):
                    rhs = (wB[:, b, nt * 512:(nt + 1) * 512] if BF16_MODE
                           else wB[:, b, :, nt * 512:(nt + 1) * 512])
                    nc.tensor.matmul(pss[nt][:], lhsT, rhs,
                                     start=(b == 0), stop=(b == k_iters - 1),
                                     perf_mode=mm_mode)
            for nt in range(N_TILES):
                ob = ob_pool.tile([128, 512], F32)
                nc.scalar.activation(ob[:], pss[nt][:], ACTF.Copy, scale=4.0)
                nc.sync.dma_start(out_ap[mt * 128:(mt + 1) * 128,
                                         nt * 512:(nt + 1) * 512], ob[:])


_NC_CACHE = None


def get_nc():
    global _NC_CACHE
    if _NC_CACHE is None:
        _NC_CACHE = build_nc()
    return _NC_CACHE


def make_in_maps(x, kernel):
    in_maps = []
    for c in range(8):
        i, j = c // COL_GROUPS, c % COL_GROUPS
        x_shard = x[i * M_CORE:(i + 1) * M_CORE, :]
        if HOST_TRANSPOSE:
            # pre-swizzle to the SBUF image [M_TILES, 128p, KS, 128m]:
            # element (mt, p, kt, m) = x[mt*128 + m, kt*128 + p]
            x_shard = x_shard.reshape(M_TILES, 128, KS, 128) \
                .transpose(0, 3, 2, 1)
        in_maps.append({
            "x": np.ascontiguousarray(x_shard),
            "w": np.ascontiguousarray(kernel[:, j * N_CORE:(j + 1) * N_CORE]),
        })
    return in_maps


def assemble(results):
    out = np.empty((M_FULL, N_FULL), dtype=np.float32)
    for c in range(8):
        i, j = c // COL_GROUPS, c % COL_GROUPS
        out[i * M_CORE:(i + 1) * M_CORE, j * N_CORE:(j + 1) * N_CORE] = \
            results[c]["out"]
    return out


def kernel(x, kernel):
    x = np.asarray(x, dtype=np.float32)
    w = np.asarray(kernel, dtype=np.float32)
    nc = get_nc()
    res = run_bass_kernel_spmd(nc, make_in_maps(x, w), list(range(8)))
    return assemble(res.results)


# revision 30
# speedup vs baseline: 1.3058x; 1.0928x over previous
"""BinaryLayer kernel for Trainium2 (8 NeuronCores).

Computes out = binarize(x) @ binarize(W), binarize(t) = where(t >= 0, 1, -1),
for x: [8192, 4096] f32, W: [4096, 4096] f32.

Sharding (2D, 8 cores as 4x2 grid): core c = (i, j) with i = c // 2 (4 row
groups of x) and j = c % 2 (2 column groups of W). Each core computes a
[2048, 2048] output block from x rows [2048*i : 2048*(i+1)] and W columns
[2048*j : 2048*(j+1)].

Per-core pipeline (everything on-device):
  1. binarize x tiles to +-0.5 in fp8e4 (one DVE tensor_scalar: (x>=0)-0.5)
  2. transpose x_bin via TensorE matmuls against an fp8 identity (PSUM), so
     the contraction dim lands on partitions; evacuate to an SBUF-resident
     x^T fp8 tensor in DoubleRow [128, 2, m] plane layout
  3. binarize W k-row blocks straight into the DoubleRow [128, 2, n] layout
  4. main matmuls in fp8 DoubleRow perf mode (K=256 per instruction),
     accumulating 4096-deep dot products in fp32 PSUM
  5. evacuate PSUM with ScalarE Copy(scale=4.0) - products were (+-0.5)^2,
     so x4 restores exact integer results - and DMA out.

All values (+-0.5 operands, 0.25*integer partial sums, x4 rescale) are exact
in fp8/fp32, so the result matches the f32 reference bit-for-bit.
"""

import os

import numpy as np

import concourse.bass as bass
import concourse.tile as tile
import concourse.mybir as mybir
from concourse import bacc
from concourse.bass_utils import run_bass_kernel_spmd
from concourse.masks import make_identity

# When set, the host hands each core its x shard in blocked-transposed
# layout [M_TILES, K, 128] (each 128-row block of x transposed), so every
# per-block DMA still carries the full contraction dim and the on-device
# TensorE transpose phase is skipped entirely.
HOST_TRANSPOSE = os.environ.get("HOST_TRANSPOSE", "0") == "1"

# When set, stationary operands use DoubleRowSwInterleave layout (software
# pre-interleaved, contiguous weight reads) instead of plain DoubleRow.
SWI = os.environ.get("SWI", "0") == "1"

# When set, use bf16 operands with plain matmuls (no DoubleRow perf mode);
# K=128 per matmul, relying on FWL for cheap weight loads.
BF16_MODE = os.environ.get("BF16_MODE", "0") == "1"

F32 = mybir.dt.float32
FP8 = mybir.dt.float8e4
BF16 = mybir.dt.bfloat16
DR = mybir.MatmulPerfMode.DoubleRow
ALU = mybir.AluOpType
ACTF = mybir.ActivationFunctionType

# Full problem shape (hardcoded; the harness always calls with these).
M_FULL, K_FULL = 8192, 4096
N_FULL = 4096

# Core grid: ROW_GROUPS x COL_GROUPS = 8. r8c1 replicates W on every core
# (maximum weight-load amortization in the matmul: 8 matmuls per LDWEIGHTS);
# r4c2 halves DMA traffic at the cost of reuse 4.
ROW_GROUPS = int(os.environ.get("ROWG", "4"))
COL_GROUPS = 8 // ROW_GROUPS
M_CORE = M_FULL // ROW_GROUPS  # 2048 rows of x per core
N_CORE = N_FULL // COL_GROUPS  # 2048 cols of W per core

M_TILES = M_CORE // 128        # 16
N_TILES = N_CORE // 512        # 4
KS = K_FULL // 128             # 32 k-subtiles of 128
KB = K_FULL // 256             # 16 DoubleRow super-blocks of 256


def binarize_half(nc, out_ap, in_ap):
    """out = (in >= 0) - 0.5  ->  +-0.5 exactly (one DVE instruction)."""
    nc.vector.tensor_scalar(out_ap, in_ap, 0.0, 0.5, ALU.is_ge, ALU.subtract)


def build_nc(loop_iters=1):
    nc = bacc.Bacc("TRN2", target_bir_lowering=False, debug=False)
    x_shape = ([M_TILES, 128, KS, 128] if HOST_TRANSPOSE
               else [M_CORE, K_FULL])
    x_ap = nc.dram_tensor("x", x_shape, F32, kind="ExternalInput").ap()
    w_ap = nc.dram_tensor("w", [K_FULL, N_CORE], F32, kind="ExternalInput").ap()
    out_ap = nc.dram_tensor("out", [M_CORE, N_CORE], F32, kind="ExternalOutput").ap()

    with tile.TileContext(nc) as tc:
        if loop_iters > 1:
            # benchmarking only: repeat the (idempotent) body on-device so
            # per-iteration time can be separated from dispatch overhead
            with tc.For_i(0, loop_iters, 1):
                kernel_body(tc, out_ap, x_ap, w_ap)
        else:
            kernel_body(tc, out_ap, x_ap, w_ap)
    nc.compile()
    return nc


def kernel_body(tc, out_ap, x_ap, w_ap):
    nc = tc.nc
    import contextlib

    with contextlib.ExitStack() as ctx:
        const_pool = ctx.enter_context(tc.tile_pool(name="const", bufs=1))
        xT_pool = ctx.enter_context(tc.tile_pool(name="xT", bufs=1))
        wB_pool = ctx.enter_context(tc.tile_pool(name="wB", bufs=1))
        xf_pool = ctx.enter_context(tc.tile_pool(name="xf", bufs=2))
        xb_pool = ctx.enter_context(tc.tile_pool(name="xb", bufs=2))
        wf_pool = ctx.enter_context(tc.tile_pool(name="wf", bufs=3))
        ob_pool = ctx.enter_context(tc.tile_pool(name="ob", bufs=4))
        ps_pool = ctx.enter_context(tc.tile_pool(name="ps", bufs=8, space="PSUM"))

        # Persistent binarized operands.
        # wB (moving operand) plane layout: element (p, b, i, n) holds W_bin
        # at contraction index k = b*256 + i*128 + p.
        # xT (stationary) layout depends on mode:
        #   plain DoubleRow:  [128, KB, 2, M_CORE], (p, b, i, m) at same k
        #   SwInterleave:     [128, KB, M_TILES, 256] - per (b, mt) the 256
        #     weight columns are [A127 B127 A126 B126 ... A0 B0] where
        #     A/B = plane 0/1 and the column index is the reversed local m.
        if BF16_MODE:
            # [p, kt, c]: value at contraction index k = kt*128 + p
            xT = xT_pool.tile([128, KS, M_CORE], BF16)
            wB = wB_pool.tile([128, KS, N_CORE], BF16)
        elif SWI:
            xT = xT_pool.tile([128, KB, M_TILES, 256], FP8)
            wB = wB_pool.tile([128, KB, 2, N_CORE], FP8)
        else:
            xT = xT_pool.tile([128, KB, 2, M_CORE], FP8)
            wB = wB_pool.tile([128, KB, 2, N_CORE], FP8)

        opd = BF16 if BF16_MODE else FP8

        def emit_w_slab(nt):
            # load + binarize W columns [nt*512, (nt+1)*512) for all k
            for kt in range(KS):
                b, i = kt // 2, kt % 2
                wf = wf_pool.tile([128, 512], F32, tag="wf")
                nc.sync.dma_start(wf[:], w_ap[kt * 128:(kt + 1) * 128,
                                              nt * 512:(nt + 1) * 512])
                full = wB[:, kt, :] if BF16_MODE else wB[:, b, i, :]
                binarize_half(nc, full[:, nt * 512:(nt + 1) * 512], wf[:])

        def emit_x_block(mt):
            # host pre-swizzled block: x_ap[mt] is [128p, KS, 128m] - the
            # exact SBUF image, so the DMA is one contiguous 2MB read
            xf = xf_pool.tile([128, KS, 128], F32, tag="xf")
            nc.sync.dma_start(xf[:], x_ap[mt])
            # kt-order (b, i) matches the xT free layout (b, i, m)
            binarize_half(nc, xT[:, :, :, mt * 128:(mt + 1) * 128],
                          xf[:].rearrange("p kt m -> p (kt m)"))

        if HOST_TRANSPOSE:
            assert not SWI and not BF16_MODE

            def emit_w_rows():
                # full-width 1MB k-rows (best DMA size)
                for kt in range(KS):
                    b, i = kt // 2, kt % 2
                    wf = wf_pool.tile([128, N_CORE], F32, tag="wfw")
                    nc.sync.dma_start(wf[:], w_ap[kt * 128:(kt + 1) * 128, :])
                    binarize_half(nc, wB[:, b, i, :], wf[:])

            order = os.environ.get("BHT_ORDER", "x01")
            if order == "wfirst":
                emit_w_rows()
                for mt in range(M_TILES):
                    emit_x_block(mt)
            else:
                # two x blocks first so early matmuls overlap the W window
                for mt in (0, 1):
                    emit_x_block(mt)
                emit_w_rows()
                for mt in range(2, M_TILES):
                    emit_x_block(mt)
        else:
            # fp8 identity for TensorE transposes; SWI uses the anti-diagonal
            # permutation so the interleaved store needs only positive strides
            ident_bf16 = const_pool.tile([128, 128], BF16)
            if SWI:
                nc.gpsimd.memset(ident_bf16[:], 0.0)
                nc.gpsimd.affine_select(
                    out=ident_bf16[:], in_=ident_bf16[:],
                    compare_op=ALU.not_equal, fill=1.0,
                    base=-127, pattern=[[1, 128]], channel_multiplier=1)
            else:
                make_identity(nc, ident_bf16[:])
            if BF16_MODE:
                ident = ident_bf16
            else:
                ident = const_pool.tile([128, 128], FP8)
                nc.vector.tensor_copy(ident[:], ident_bf16[:])

            # ---- Phase X: load + binarize + transpose x ----
            x_chunk = 2048
            for mt in range(M_TILES):
                xb = xb_pool.tile([128, K_FULL], opd)
                for h in range(0, K_FULL, x_chunk):
                    xf = xf_pool.tile([128, x_chunk], F32)
                    nc.sync.dma_start(xf[:], x_ap[mt * 128:(mt + 1) * 128,
                                                   h:h + x_chunk])
                    binarize_half(nc, xb[:, h:h + x_chunk], xf[:])
                # transpose 32 k-subtiles in groups of 4 (one PSUM bank per group)
                for kg in range(KS // 4):
                    ps = ps_pool.tile([128, 512], F32, tag="ps")
                    for t in range(4):
                        kt = kg * 4 + t
                        nc.tensor.matmul(ps[:, t * 128:(t + 1) * 128],
                                         xb[:, kt * 128:(kt + 1) * 128],
                                         ident[:], start=True, stop=True)
                    if BF16_MODE:
                        dst = xT[:, 4 * kg:4 * kg + 4,
                                 mt * 128:(mt + 1) * 128]
                    elif SWI:
                        # psum col t*128+c holds x_bin[127-c, k=kt*128+p];
                        # scatter to interleaved j = 2c + (t%2) of block
                        # (b = 2kg + t//2, mt)
                        dst = xT[:, 2 * kg:2 * kg + 2, mt, :].rearrange(
                            "p b (m two) -> p b two m", two=2)
                    else:
                        dst = xT[:, 2 * kg:2 * kg + 2, :,
                                 mt * 128:(mt + 1) * 128]
                    if kg % 2 == 0:
                        nc.vector.tensor_copy(dst, ps[:])
                    else:
                        nc.scalar.activation(dst, ps[:], ACTF.Copy)

        # (W slab loading is emitted around phase X; see emit_w_slab below)

        if not HOST_TRANSPOSE:
            # W after x: full-width 1MB k-rows
            for kt in range(KS):
                b, i = kt // 2, kt % 2
                wf = wf_pool.tile([128, N_CORE], F32, tag="wfw")
                nc.sync.dma_start(wf[:], w_ap[kt * 128:(kt + 1) * 128, :])
                dstw = wB[:, kt, :] if BF16_MODE else wB[:, b, i, :]
                binarize_half(nc, dstw, wf[:])

        # ---- Main matmuls: mt-outer, b-mid, nt-inner (each stationary
        # ---- operand serves 4 consecutive matmuls)
        mm_mode = (None if BF16_MODE else
                   (mybir.MatmulPerfMode.DoubleRowSwInterleave if SWI else DR))
        k_iters = KS if BF16_MODE else KB
        for mt in range(M_TILES):
            pss = [ps_pool.tile([128, 512], F32, name=f"ps_{mt}_{nt}",
                                tag="ps") for nt in range(N_TILES)]
            for b in range(k_iters):
                if BF16_MODE:
                    lhsT = xT[:, b, mt * 128:(mt + 1) * 128]
                elif SWI:
                    lhsT = xT[:, b, mt, :]
                else:
                    lhsT = xT[:, b, :, mt * 128:(mt + 1) * 128]
                for nt in range(N_TILES):
                    rhs = (wB[:, b, nt * 512:(nt + 1) * 512] if BF16_MODE
                           else wB[:, b, :, nt * 512:(nt + 1) * 512])
                    nc.tensor.matmul(pss[nt][:], lhsT, rhs,
                                     start=(b == 0), stop=(b == k_iters - 1),
                                     perf_mode=mm_mode)
            for nt in range(N_TILES):
                ob = ob_pool.tile([128, 512], F32)
                nc.scalar.activation(ob[:], pss[nt][:], ACTF.Copy, scale=4.0)
                nc.sync.dma_start(out_ap[mt * 128:(mt + 1) * 128,
                                         nt * 512:(nt + 1) * 512], ob[:])


_NC_CACHE = None


def get_nc():
    global _NC_CACHE
    if _NC_CACHE is None:
        _NC_CACHE = build_nc()
    return _NC_CACHE


def make_in_maps(x, kernel):
    in_maps = []
    for c in range(8):
        i, j = c // COL_GROUPS, c % COL_GROUPS
        x_shard = x[i * M_CORE:(i + 1) * M_CORE, :]
        if HOST_TRANSPOSE:
            # pre-swizzle to the SBUF image [M_TILES, 128p, KS, 128m]:
            # element (mt, p, kt, m) = x[mt*128 + m, kt*128 + p]
            x_shard = x_shard.reshape(M_TILES, 128, KS, 128) \
                .transpose(0, 3, 2, 1)
        in_maps.append({
            "x": np.ascontiguousarray(x_shard),
            "w": np.ascontiguousarray(kernel[:, j * N_CORE:(j + 1) * N_CORE]),
        })
    return in_maps


def assemble(results):
    out = np.empty((M_FULL, N_FULL), dtype=np.float32)
    for c in range(8):
        i, j = c // COL_GROUPS, c % COL_GROUPS
        out[i * M_CORE:(i + 1) * M_CORE, j * N_CORE:(j + 1) * N_CORE] = \
            results[c]["out"]
    return out


def kernel(x, kernel):
    x = np.asarray(x, dtype=np.float32)
    w = np.asarray(kernel, dtype=np.float32)
    nc = get_nc()
    res = run_bass_kernel_spmd(nc, make_in_maps(x, w), list(range(8)))
    return assemble(res.results)
